# revision 6
# baseline (speedup 1.0000x reference)
"""RNN LM (embedding -> tanh RNN -> FC -> log_softmax) on 8 trn2 NeuronCores.

Sharding: data-parallel over batch (16 batches -> 2 per core), params
replicated. Each core runs the full 256-step recurrence for its 2 batch rows,
then computes its [512, 32000] slice of log_softmax(hs @ W_fc + b_fc) with a
core-local logsumexp (no collectives).

Kernel structure (per core):
  1. Gather embedding rows for the 512 local tokens (indirect DMA), PE
     transpose into X^T [100, 512] (time-major columns: col = 2*s + b).
  2. One matmul A = [W_ih; b_ih; b_hh]^T @ [X^T; 1; 1] -> PSUM [100, 512]
     (input-to-hidden contributions + both biases for every step at once).
  3. Recurrence, 256 steps: matmul-accumulate W_hh^T h_{s-1} onto A's columns
     in PSUM, then Tanh (ScalarE) -> hs tiles in SBUF (hidden on partitions).
  4. FC + log_softmax per 128-token tile: pass A computes logits chunks
     (float32r matmuls, bias folded via ones-row in lhsT / b_fc row in rhs)
     and exp-accumulates per-token sums (ScalarE Exp accum_out), then
     lse = Ln(sum); pass B recomputes logits chunks and writes
     logits - lse (VectorE tensor_scalar_sub) staged into 1MB output DMAs.
"""

import sys
from contextlib import ExitStack

import numpy as np

try:
    import concourse.bass as bass
except ImportError:  # pragma: no cover - fallback if site path is missing
    sys.path.insert(0, "/opt/trn_rl_repo")
    import concourse.bass as bass

import concourse.tile as tile
from concourse import bacc, mybir
from concourse.bass_utils import run_bass_kernel_spmd
from concourse.masks import make_identity

V, E, H = 32000, 100, 100
B, S = 16, 256
NCORES = 8
BL = B // NCORES  # local batches per core
TOK = BL * S  # local tokens
SEG = 2  # recurrence segments
SS = S // SEG  # steps per segment
NV = 500  # vocab chunk (<=512 fp32 PSUM bank)
NCH = V // NV  # 64 chunks
GRP = 4  # chunks per staged output DMA (1 MB)

f32 = mybir.dt.float32
f32r = mybir.dt.float32r
i32 = mybir.dt.int32
AFT = mybir.ActivationFunctionType
AX = mybir.AxisListType


def _emit(ctx, tc, dr):
    nc = tc.nc
    singles = ctx.enter_context(tc.tile_pool(name="singles", bufs=1))
    gat = ctx.enter_context(tc.tile_pool(name="gat", bufs=2))
    tp_ps = ctx.enter_context(tc.tile_pool(name="tp_ps", bufs=2, space="PSUM"))
    pa_ps = ctx.enter_context(tc.tile_pool(name="pa_ps", bufs=1, space="PSUM"))
    fc_ps = ctx.enter_context(tc.tile_pool(name="fc_ps", bufs=5, space="PSUM"))
    expp = ctx.enter_context(tc.tile_pool(name="expp", bufs=1))
    stats_p = ctx.enter_context(tc.tile_pool(name="stats_p", bufs=2))
    small = ctx.enter_context(tc.tile_pool(name="small", bufs=4))
    stage = ctx.enter_context(tc.tile_pool(name="stage", bufs=3))

    # --- load params / constants ---
    Wfc_sb = singles.tile([E + 1, V], f32r)
    for k in range(16):
        nc.sync.dma_start(
            Wfc_sb[:, k * 2000 : (k + 1) * 2000],
            dr["Wfc_aug"][:, k * 2000 : (k + 1) * 2000].bitcast(f32r),
        )
    Waug_sb = singles.tile([E + 2, H], f32)
    nc.sync.dma_start(Waug_sb[:], dr["W_aug"][:])
    Whh_sb = singles.tile([H, H], f32)
    nc.sync.dma_start(Whh_sb[:], dr["W_hh"][:])
    idx_sb = singles.tile([128, 4], i32)
    nc.sync.dma_start(idx_sb[:], dr["xt_idx"][:])
    ident = singles.tile([128, 128], f32)
    make_identity(nc, ident[:])

    # Rows E..E+1 must be ones (bias rows); memset the whole tile, rows 0..E-1
    # get overwritten by the gather/transpose copies below.
    XaugT = singles.tile([E + 2, TOK], f32)
    nc.vector.memset(XaugT[:], 1.0)

    # Row H must be ones (FC bias row); rows 0..H-1 overwritten by tanh/h0T.
    hs = []
    for q in range(SEG):
        t = singles.tile([H + 1, 2 + 2 * SS], f32, tag=f"hs{q}")
        nc.vector.memset(t[:], 1.0)
        hs.append(t)
    nc.sync.dma_start(hs[0][0:H, 0:2], dr["h0T"][:])

    # --- embedding gather + transpose into X^T ---
    for j in range(4):
        g = gat.tile([128, E], f32)
        nc.gpsimd.indirect_dma_start(
            out=g[:],
            out_offset=None,
            in_=dr["emb"][:],
            in_offset=bass.IndirectOffsetOnAxis(ap=idx_sb[:, j : j + 1], axis=0),
        )
        tp = tp_ps.tile([E, 128], f32)
        nc.tensor.transpose(tp[:], g[:], ident[:])
        nc.vector.tensor_copy(XaugT[0:E, j * 128 : (j + 1) * 128], tp[:])

    # --- A = W_aug^T @ X_aug (input contributions + biases, all steps) ---
    psA = pa_ps.tile([H, 2 * S], f32)
    nc.tensor.matmul(psA[:], Waug_sb[:], XaugT[:], start=True, stop=True)

    # --- recurrence: h_s = tanh(A_s + W_hh^T h_{s-1}) ---
    for s in range(S):
        q, sl = divmod(s, SS)
        if s == 0:
            rhs = hs[0][0:H, 0:2]
        else:
            pq, psl = divmod(s - 1, SS)
            rhs = hs[pq][0:H, 2 + 2 * psl : 4 + 2 * psl]
        nc.tensor.matmul(
            psA[:, 2 * s : 2 * s + 2],
            Whh_sb[:],
            rhs,
            start=False,
            stop=True,
            skip_group_check=True,
        )
        nc.scalar.activation(
            hs[q][0:H, 2 + 2 * sl : 4 + 2 * sl], psA[:, 2 * s : 2 * s + 2], AFT.Tanh
        )

    nc.sync.dma_start(dr["h_lastT"][:], hs[SEG - 1][0:H, 2 * SS : 2 * SS + 2])

    # f32 -> f32r conversion of hs for the FC matmuls (walrus requires f32r
    # matmul inputs to be produced as f32r).
    hsr = []
    for q in range(SEG):
        t = singles.tile([H + 1, 2 * SS], f32r, tag=f"hsr{q}")
        nc.vector.tensor_copy(t[:], hs[q][0 : H + 1, 2 : 2 + 2 * SS])
        hsr.append(t)

    # --- FC + log_softmax, 128-token tiles ---
    for h in range(SEG):
        for b in range(BL):
            lhsT = hsr[h][0 : H + 1, b : 2 * SS : 2]
            row0 = b * S + h * SS
            # pass A: exp-sums
            stats = stats_p.tile([128, NCH], f32)
            for g in range(NCH):
                ps = fc_ps.tile([128, NV], f32, tag="fc")
                nc.tensor.matmul(
                    ps[:],
                    lhsT,
                    Wfc_sb[:, g * NV : (g + 1) * NV],
                    start=True,
                    stop=True,
                )
                ex = expp.tile([128, NV], f32)
                nc.scalar.activation(
                    ex[:], ps[:], AFT.Exp, accum_out=stats[:, g : g + 1]
                )
            esum = small.tile([128, 1], f32)
            nc.vector.reduce_sum(esum[:], stats[:], axis=AX.X)
            lse = small.tile([128, 1], f32)
            nc.scalar.activation(lse[:], esum[:], AFT.Ln)
            # pass B: logits - lse, staged output
            for grp in range(NCH // GRP):
                stg = stage.tile([128, GRP * NV], f32)
                for gg in range(GRP):
                    g = grp * GRP + gg
                    ps = fc_ps.tile([128, NV], f32, tag="fc")
                    nc.tensor.matmul(
                        ps[:],
                        lhsT,
                        Wfc_sb[:, g * NV : (g + 1) * NV],
                        start=True,
                        stop=True,
                    )
                    nc.vector.tensor_scalar_sub(
                        stg[:, gg * NV : (gg + 1) * NV], ps[:], lse[:, 0:1]
                    )
                nc.sync.dma_start(
                    dr["out_lp"][
                        row0 : row0 + 128, grp * GRP * NV : (grp + 1) * GRP * NV
                    ],
                    stg[:],
                )


def build_module():
    nc = bacc.Bacc("TRN2", target_bir_lowering=False, debug=False)
    dr = {
        "xt_idx": nc.dram_tensor("xt_idx", [128, 4], i32, kind="ExternalInput").ap(),
        "h0T": nc.dram_tensor("h0T", [H, BL], f32, kind="ExternalInput").ap(),
        "W_aug": nc.dram_tensor("W_aug", [E + 2, H], f32, kind="ExternalInput").ap(),
        "W_hh": nc.dram_tensor("W_hh", [H, H], f32, kind="ExternalInput").ap(),
        "Wfc_aug": nc.dram_tensor(
            "Wfc_aug", [E + 1, V], f32, kind="ExternalInput"
        ).ap(),
        "emb": nc.dram_tensor("emb", [V, E], f32, kind="ExternalInput").ap(),
        "out_lp": nc.dram_tensor("out_lp", [TOK, V], f32, kind="ExternalOutput").ap(),
        "h_lastT": nc.dram_tensor(
            "h_lastT", [H, BL], f32, kind="ExternalOutput"
        ).ap(),
    }
    with tile.TileContext(nc) as tc:
        with ExitStack() as ctx:
            _emit(ctx, tc, dr)
    nc.compile()
    return nc


def make_in_maps(inputs):
    x = np.asarray(inputs["x"]).astype(np.int32)  # [16, 256]
    hidden = np.asarray(inputs["hidden"], dtype=np.float32)  # [16, 100]
    emb = np.ascontiguousarray(np.asarray(inputs["emb"], dtype=np.float32))
    W_ih = np.asarray(inputs["W_ih"], dtype=np.float32)
    W_hh = np.ascontiguousarray(np.asarray(inputs["W_hh"], dtype=np.float32))
    b_ih = np.asarray(inputs["b_ih"], dtype=np.float32)
    b_hh = np.asarray(inputs["b_hh"], dtype=np.float32)
    W_fc = np.asarray(inputs["W_fc"], dtype=np.float32)
    b_fc = np.asarray(inputs["b_fc"], dtype=np.float32)

    W_aug = np.ascontiguousarray(np.vstack([W_ih, b_ih[None, :], b_hh[None, :]]))
    Wfc_aug = np.ascontiguousarray(np.vstack([W_fc, b_fc[None, :]]))

    in_maps = []
    for c in range(NCORES):
        xb = x[BL * c : BL * (c + 1)]  # [2, 256]
        ti = np.ascontiguousarray(xb.T).reshape(TOK)  # time-major: i = 2*s + b
        xt_idx = np.ascontiguousarray(ti.reshape(4, 128).T)  # [p, j] = ti[128j + p]
        h0T = np.ascontiguousarray(hidden[BL * c : BL * (c + 1)].T)  # [100, 2]
        in_maps.append(
            {
                "xt_idx": xt_idx,
                "h0T": h0T,
                "W_aug": W_aug,
                "W_hh": W_hh,
                "Wfc_aug": Wfc_aug,
                "emb": emb,
            }
        )
    return in_maps


def assemble_outputs(results):
    log_probs = np.concatenate(
        [results[c]["out_lp"].reshape(BL, S, V) for c in range(NCORES)], axis=0
    )
    h_last = np.concatenate(
        [results[c]["h_lastT"].T for c in range(NCORES)], axis=0
    )
    return log_probs, h_last


_NC_CACHE = None


def kernel(**inputs):
    global _NC_CACHE
    if _NC_CACHE is None:
        _NC_CACHE = build_module()
    in_maps = make_in_maps(inputs)
    res = run_bass_kernel_spmd(_NC_CACHE, in_maps, core_ids=list(range(NCORES)))
    return assemble_outputs(res.results)


# revision 7
# speedup vs baseline: 1.5545x; 1.5545x over previous
"""RNN LM (embedding -> tanh RNN -> FC -> log_softmax) on 8 trn2 NeuronCores.

Sharding: data-parallel over batch (16 batches -> 2 per core), params
replicated. Each core runs the full 256-step recurrence for its 2 batch rows,
then computes its [512, 32000] slice of log_softmax(hs @ W_fc + b_fc) with a
core-local logsumexp (no collectives).

Kernel structure (per core):
  1. Gather embedding rows for the 512 local tokens (indirect DMA), PE
     transpose into X^T [100, 512] (time-major columns: col = 2*s + b).
  2. One matmul A = [W_ih; b_ih; b_hh]^T @ [X^T; 1; 1] -> PSUM [100, 512]
     (input-to-hidden contributions + both biases for every step at once).
  3. Recurrence, 256 steps: matmul-accumulate W_hh^T h_{s-1} onto A's columns
     in PSUM, then Tanh (ScalarE) -> hs tiles in SBUF (hidden on partitions).
  4. FC + log_softmax per 128-token tile in bf16 (hs and W_fc converted
     on-device): pass A computes logits into 2-bank PSUM groups and
     exp-accumulates per-token sums (ScalarE Exp accum_out), lse = Ln(sum);
     pass B recomputes logits and writes logits - lse (VectorE
     tensor_scalar_sub) staged into ~1MB output DMAs.

DMA notes: big transfers (W_fc load, output stores) go through SWDGE
(nc.gpsimd) so each dma_start's descriptors spread across all 16 SDMA
engines; the HWDGE ring (nc.sync) serializes one engine per transfer.
Small input DMAs are issued first so the recurrence isn't stuck behind
the W_fc load.
"""

import sys
from contextlib import ExitStack

import numpy as np

try:
    import concourse.bass as bass
except ImportError:  # pragma: no cover - fallback if site path is missing
    sys.path.insert(0, "/opt/trn_rl_repo")
    import concourse.bass as bass

import concourse.tile as tile
from concourse import bacc, mybir
from concourse.bass_utils import run_bass_kernel_spmd
from concourse.masks import make_identity

V, E, H = 32000, 100, 100
B, S = 16, 256
NCORES = 8
BL = B // NCORES  # local batches per core
TOK = BL * S  # local tokens
SEG = 2  # recurrence segments
SS = S // SEG  # steps per segment
GW = 1024  # psum group width (2 banks)
NG = (V + GW - 1) // GW  # 32 groups per m-tile (last one is 256 wide)
SW = 2048  # staging width (2 groups, ~1MB DMA)
NST = (V + SW - 1) // SW  # 16 staged output DMAs per m-tile

f32 = mybir.dt.float32
f32r = mybir.dt.float32r
bf16 = mybir.dt.bfloat16
i32 = mybir.dt.int32
AFT = mybir.ActivationFunctionType
AX = mybir.AxisListType


def _emit(ctx, tc, dr):
    nc = tc.nc
    singles = ctx.enter_context(tc.tile_pool(name="singles", bufs=1))
    gat = ctx.enter_context(tc.tile_pool(name="gat", bufs=2))
    wfc_ld = ctx.enter_context(tc.tile_pool(name="wfc_ld", bufs=2))
    tp_ps = ctx.enter_context(tc.tile_pool(name="tp_ps", bufs=1, space="PSUM"))
    pa_ps = ctx.enter_context(tc.tile_pool(name="pa_ps", bufs=1, space="PSUM"))
    fc_ps = ctx.enter_context(tc.tile_pool(name="fc_ps", bufs=3, space="PSUM"))
    expp = ctx.enter_context(tc.tile_pool(name="expp", bufs=2))
    stats_p = ctx.enter_context(tc.tile_pool(name="stats_p", bufs=2))
    small = ctx.enter_context(tc.tile_pool(name="small", bufs=4))
    stage = ctx.enter_context(tc.tile_pool(name="stage", bufs=3))

    # --- small params first (sync HWDGE ring is FIFO; keep it short) ---
    idx_sb = singles.tile([128, 4], i32)
    nc.sync.dma_start(idx_sb[:], dr["xt_idx"][:])
    Waug_sb = singles.tile([E + 2, H], f32)
    nc.sync.dma_start(Waug_sb[:], dr["W_aug"][:])
    Whh_sb = singles.tile([H, H], f32)
    nc.sync.dma_start(Whh_sb[:], dr["W_hh"][:])
    ident = singles.tile([128, 128], f32)
    make_identity(nc, ident[:])

    # Rows E..E+1 must be ones (bias rows); memset the whole tile, rows 0..E-1
    # get overwritten by the gather/transpose copies below.
    XaugT = singles.tile([E + 2, TOK], f32)
    nc.vector.memset(XaugT[:], 1.0)

    # Row H must be ones (FC bias row); rows 0..H-1 overwritten by tanh/h0T.
    hs = []
    for q in range(SEG):
        t = singles.tile([H + 1, 2 + 2 * SS], f32, tag=f"hs{q}")
        nc.vector.memset(t[:], 1.0)
        hs.append(t)
    nc.sync.dma_start(hs[0][0:H, 0:2], dr["h0T"][:])

    # --- embedding gather + transpose into X^T ---
    for j in range(4):
        g = gat.tile([128, E], f32)
        nc.gpsimd.indirect_dma_start(
            out=g[:],
            out_offset=None,
            in_=dr["emb"][:],
            in_offset=bass.IndirectOffsetOnAxis(ap=idx_sb[:, j : j + 1], axis=0),
        )
        tp = tp_ps.tile([E, 128], f32)
        nc.tensor.transpose(tp[:], g[:], ident[:])
        nc.vector.tensor_copy(XaugT[0:E, j * 128 : (j + 1) * 128], tp[:])

    # --- A = W_aug^T @ X_aug (input contributions + biases, all steps) ---
    psA = pa_ps.tile([H, 2 * S], f32)
    nc.tensor.matmul(psA[:], Waug_sb[:], XaugT[:], start=True, stop=True)

    # --- W_fc load (SWDGE) + on-device f32 -> bf16 conversion ---
    Wfcb = singles.tile([E + 1, V], bf16)
    for k in range(16):
        sl = slice(k * 2000, (k + 1) * 2000)
        w = wfc_ld.tile([E + 1, 2000], f32)
        nc.gpsimd.dma_start(w[:], dr["Wfc_aug"][:, sl])
        nc.vector.tensor_copy(Wfcb[:, sl], w[:])

    # --- recurrence: h_s = tanh(A_s + W_hh^T h_{s-1}) ---
    for s in range(S):
        q, sl = divmod(s, SS)
        if s == 0:
            rhs = hs[0][0:H, 0:2]
        else:
            pq, psl = divmod(s - 1, SS)
            rhs = hs[pq][0:H, 2 + 2 * psl : 4 + 2 * psl]
        nc.tensor.matmul(
            psA[:, 2 * s : 2 * s + 2],
            Whh_sb[:],
            rhs,
            start=False,
            stop=True,
            skip_group_check=True,
        )
        nc.scalar.activation(
            hs[q][0:H, 2 + 2 * sl : 4 + 2 * sl], psA[:, 2 * s : 2 * s + 2], AFT.Tanh
        )

    nc.sync.dma_start(dr["h_lastT"][:], hs[SEG - 1][0:H, 2 * SS : 2 * SS + 2])

    # f32 -> bf16 conversion of hs for the FC matmuls.
    hsb = []
    for q in range(SEG):
        t = singles.tile([H + 1, 2 * SS], bf16, tag=f"hsb{q}")
        nc.vector.tensor_copy(t[:], hs[q][0 : H + 1, 2 : 2 + 2 * SS])
        hsb.append(t)

    # --- FC + log_softmax, 128-token tiles ---
    for h in range(SEG):
        for b in range(BL):
            lhsT = hsb[h][0 : H + 1, b : 2 * SS : 2]  # [101, 128]
            row0 = b * S + h * SS
            # pass A: exp-sums over 1024-wide psum groups
            stats = stats_p.tile([128, NG], f32)
            for j in range(NG):
                w = min(GW, V - j * GW)
                ps = fc_ps.tile([128, GW], f32, tag="fc")
                for k in range(0, w, 512):
                    kw = min(512, w - k)
                    nc.tensor.matmul(
                        ps[:, k : k + kw],
                        lhsT,
                        Wfcb[:, j * GW + k : j * GW + k + kw],
                        start=True,
                        stop=True,
                    )
                ex = expp.tile([128, GW], f32)
                nc.scalar.activation(
                    ex[:, 0:w], ps[:, 0:w], AFT.Exp, accum_out=stats[:, j : j + 1]
                )
            esum = small.tile([128, 1], f32)
            nc.vector.reduce_sum(esum[:], stats[:], axis=AX.X)
            lse = small.tile([128, 1], f32)
            nc.scalar.activation(lse[:], esum[:], AFT.Ln)
            # pass B: logits - lse, staged ~1MB output DMAs
            for g in range(NST):
                sw = min(SW, V - g * SW)
                stg = stage.tile([128, SW], f32)
                for jj in range((sw + GW - 1) // GW):
                    off = jj * GW
                    w = min(GW, sw - off)
                    ps = fc_ps.tile([128, GW], f32, tag="fc")
                    for k in range(0, w, 512):
                        kw = min(512, w - k)
                        nc.tensor.matmul(
                            ps[:, k : k + kw],
                            lhsT,
                            Wfcb[:, g * SW + off + k : g * SW + off + k + kw],
                            start=True,
                            stop=True,
                        )
                    nc.vector.tensor_scalar_sub(
                        stg[:, off : off + w], ps[:, 0:w], lse[:, 0:1]
                    )
                nc.gpsimd.dma_start(
                    dr["out_lp"][row0 : row0 + 128, g * SW : g * SW + sw],
                    stg[:, 0:sw],
                )


def build_module():
    nc = bacc.Bacc("TRN2", target_bir_lowering=False, debug=False)
    dr = {
        "xt_idx": nc.dram_tensor("xt_idx", [128, 4], i32, kind="ExternalInput").ap(),
        "h0T": nc.dram_tensor("h0T", [H, BL], f32, kind="ExternalInput").ap(),
        "W_aug": nc.dram_tensor("W_aug", [E + 2, H], f32, kind="ExternalInput").ap(),
        "W_hh": nc.dram_tensor("W_hh", [H, H], f32, kind="ExternalInput").ap(),
        "Wfc_aug": nc.dram_tensor(
            "Wfc_aug", [E + 1, V], f32, kind="ExternalInput"
        ).ap(),
        "emb": nc.dram_tensor("emb", [V, E], f32, kind="ExternalInput").ap(),
        "out_lp": nc.dram_tensor("out_lp", [TOK, V], f32, kind="ExternalOutput").ap(),
        "h_lastT": nc.dram_tensor(
            "h_lastT", [H, BL], f32, kind="ExternalOutput"
        ).ap(),
    }
    with tile.TileContext(nc) as tc:
        with ExitStack() as ctx:
            _emit(ctx, tc, dr)
    nc.compile()
    return nc


def make_in_maps(inputs):
    x = np.asarray(inputs["x"]).astype(np.int32)  # [16, 256]
    hidden = np.asarray(inputs["hidden"], dtype=np.float32)  # [16, 100]
    emb = np.ascontiguousarray(np.asarray(inputs["emb"], dtype=np.float32))
    W_ih = np.asarray(inputs["W_ih"], dtype=np.float32)
    W_hh = np.ascontiguousarray(np.asarray(inputs["W_hh"], dtype=np.float32))
    b_ih = np.asarray(inputs["b_ih"], dtype=np.float32)
    b_hh = np.asarray(inputs["b_hh"], dtype=np.float32)
    W_fc = np.asarray(inputs["W_fc"], dtype=np.float32)
    b_fc = np.asarray(inputs["b_fc"], dtype=np.float32)

    W_aug = np.ascontiguousarray(np.vstack([W_ih, b_ih[None, :], b_hh[None, :]]))
    Wfc_aug = np.ascontiguousarray(np.vstack([W_fc, b_fc[None, :]]))

    in_maps = []
    for c in range(NCORES):
        xb = x[BL * c : BL * (c + 1)]  # [2, 256]
        ti = np.ascontiguousarray(xb.T).reshape(TOK)  # time-major: i = 2*s + b
        xt_idx = np.ascontiguousarray(ti.reshape(4, 128).T)  # [p, j] = ti[128j + p]
        h0T = np.ascontiguousarray(hidden[BL * c : BL * (c + 1)].T)  # [100, 2]
        in_maps.append(
            {
                "xt_idx": xt_idx,
                "h0T": h0T,
                "W_aug": W_aug,
                "W_hh": W_hh,
                "Wfc_aug": Wfc_aug,
                "emb": emb,
            }
        )
    return in_maps


def assemble_outputs(results):
    log_probs = np.concatenate(
        [results[c]["out_lp"].reshape(BL, S, V) for c in range(NCORES)], axis=0
    )
    h_last = np.concatenate(
        [results[c]["h_lastT"].T for c in range(NCORES)], axis=0
    )
    return log_probs, h_last


_NC_CACHE = None


def kernel(**inputs):
    global _NC_CACHE
    if _NC_CACHE is None:
        _NC_CACHE = build_module()
    in_maps = make_in_maps(inputs)
    res = run_bass_kernel_spmd(_NC_CACHE, in_maps, core_ids=list(range(NCORES)))
    return assemble_outputs(res.results)


# revision 11
# speedup vs baseline: 2.2008x; 1.4158x over previous
"""RNN LM (embedding -> tanh RNN -> FC -> log_softmax) on 8 trn2 NeuronCores.

Sharding: data-parallel over batch (16 batches -> 2 per core), params
replicated. Each core runs the full 256-step recurrence for its 2 batch rows,
then computes its [512, 32000] slice of log_softmax(hs @ W_fc + b_fc) with a
core-local logsumexp (no collectives).

Kernel structure (per core):
  1. Gather embedding rows for the 512 local tokens (indirect DMA), PE
     transpose into X^T [100, 512] (time-major columns: col = 2*s + b).
  2. One matmul A = [W_ih; b_ih; b_hh]^T @ [X^T; 1; 1] -> PSUM [100, 512]
     (input-to-hidden contributions + both biases for every step at once).
  3. Recurrence, 256 steps: f32r matmul-accumulate W_hh^T h_{s-1} onto A's
     columns in PSUM, then Tanh (ScalarE) -> hs tiles in SBUF.
  4. FC + log_softmax per 128-token tile in bf16 (hs and W_fc converted
     on-device): pass A computes logits into 2-bank PSUM groups and
     exp-accumulates per-token sums (ScalarE Exp accum_out), lse = Ln(sum);
     pass B recomputes logits and writes logits - lse (VectorE
     tensor_scalar_sub) staged into ~2MB output DMAs.

Scheduling notes:
  - Big transfers (W_fc load, output stores) go through SWDGE (nc.gpsimd);
    per-DMA throughput is ~1 SDMA engine, so the W_fc load is split into many
    chunks with a deep scratch pool to keep several in flight.
  - The embedding gathers share the SWDGE ring with the W_fc load; an
    explicit dep makes the gathers hit the ring first (they gate the whole
    recurrence).
  - XaugT psum->sbuf copies run on ScalarE so the VectorE stream (W_fc
    casts) never waits on the gather/transpose path.
"""

import sys
from contextlib import ExitStack

import numpy as np

try:
    import concourse.bass as bass
except ImportError:  # pragma: no cover - fallback if site path is missing
    sys.path.insert(0, "/opt/trn_rl_repo")
    import concourse.bass as bass

import concourse.tile as tile
from concourse import bacc, mybir
from concourse.bass_utils import run_bass_kernel_spmd
from concourse.masks import make_identity
from concourse.tile import add_dep_helper

V, E, H = 32000, 100, 100
B, S = 16, 256
NCORES = 8
BL = B // NCORES  # local batches per core
TOK = BL * S  # local tokens
SEG = 2  # recurrence segments
SS = S // SEG  # steps per segment
GW = 1024  # psum group width (2 banks)
NG = (V + GW - 1) // GW  # 32 groups per m-tile (last one is 256 wide)
SW = 4096  # staging width (4 groups, ~2MB DMA)
NST = (V + SW - 1) // SW  # 8 staged output DMAs per m-tile
WCH = 1000  # W_fc load chunk width
NWCH = V // WCH  # 32 load chunks

f32 = mybir.dt.float32
f32r = mybir.dt.float32r
bf16 = mybir.dt.bfloat16
i32 = mybir.dt.int32
AFT = mybir.ActivationFunctionType
AX = mybir.AxisListType


def _emit(ctx, tc, dr):
    nc = tc.nc
    singles = ctx.enter_context(tc.tile_pool(name="singles", bufs=1))
    gat = ctx.enter_context(tc.tile_pool(name="gat", bufs=2))
    wfc_ld = ctx.enter_context(tc.tile_pool(name="wfc_ld", bufs=8))
    tp_ps = ctx.enter_context(tc.tile_pool(name="tp_ps", bufs=1, space="PSUM"))
    pa_ps = ctx.enter_context(tc.tile_pool(name="pa_ps", bufs=1, space="PSUM"))
    fc_ps = ctx.enter_context(tc.tile_pool(name="fc_ps", bufs=3, space="PSUM"))
    expp = ctx.enter_context(tc.tile_pool(name="expp", bufs=2))
    stats_p = ctx.enter_context(tc.tile_pool(name="stats_p", bufs=2))
    small = ctx.enter_context(tc.tile_pool(name="small", bufs=4))
    stage = ctx.enter_context(tc.tile_pool(name="stage", bufs=3))

    # --- small params first (sync HWDGE ring is FIFO; keep it short) ---
    idx_sb = singles.tile([128, 4], i32)
    nc.sync.dma_start(idx_sb[:], dr["xt_idx"][:])
    Waug_sb = singles.tile([E + 2, H], f32)
    nc.sync.dma_start(Waug_sb[:], dr["W_aug"][:])
    Whh_sb = singles.tile([H, H], f32r)
    nc.sync.dma_start(Whh_sb[:], dr["W_hh"].bitcast(f32r))
    ident = singles.tile([128, 128], f32)
    make_identity(nc, ident[:])

    # Rows E..E+1 must be ones (bias rows); memset the whole tile, rows 0..E-1
    # get overwritten by the gather/transpose copies below.
    XaugT = singles.tile([E + 2, TOK], f32)
    nc.vector.memset(XaugT[:], 1.0)

    # Recurrence state tiles (f32r, hidden on partitions, time-major columns).
    # The FC bias ones-row lives in the bf16 hsb tiles below, so no memset of
    # f32r memory is needed (walrus rejects f32r MEMSET).
    hs = []
    for q in range(SEG):
        t = singles.tile([H, 2 + 2 * SS], f32r, tag=f"hs{q}")
        hs.append(t)
    nc.sync.dma_start(hs[0][0:H, 0:2], dr["h0T"].bitcast(f32r))

    # --- embedding gather + transpose into X^T (copies on ScalarE) ---
    last_gather = None
    for j in range(4):
        g = gat.tile([128, E], f32)
        last_gather = nc.gpsimd.indirect_dma_start(
            out=g[:],
            out_offset=None,
            in_=dr["emb"][:],
            in_offset=bass.IndirectOffsetOnAxis(ap=idx_sb[:, j : j + 1], axis=0),
        )
        tp = tp_ps.tile([E, 128], f32)
        nc.tensor.transpose(tp[:], g[:], ident[:])
        nc.scalar.copy(XaugT[0:E, j * 128 : (j + 1) * 128], tp[:])

    # --- W_fc load (SWDGE, many chunks in flight) + f32 -> bf16 cast ---
    Wfcb = singles.tile([E + 1, V], bf16)
    first_chunk = None
    for k in range(NWCH):
        sl = slice(k * WCH, (k + 1) * WCH)
        w = wfc_ld.tile([E + 1, WCH], f32)
        inst = nc.gpsimd.dma_start(w[:], dr["Wfc_aug"][:, sl])
        if first_chunk is None:
            first_chunk = inst
        nc.vector.tensor_copy(Wfcb[:, sl], w[:])
    # Gathers gate the recurrence; make them hit the SWDGE ring first.
    add_dep_helper(
        first_chunk.ins, last_gather.ins, sync=False, reason="gathers first"
    )

    # --- A = W_aug^T @ X_aug (input contributions + biases, all steps) ---
    psA = pa_ps.tile([H, 2 * S], f32)
    nc.tensor.matmul(psA[:], Waug_sb[:], XaugT[:], start=True, stop=True)

    # --- recurrence: h_s = tanh(A_s + W_hh^T h_{s-1}) ---
    for s in range(S):
        q, sl = divmod(s, SS)
        if s == 0:
            rhs = hs[0][0:H, 0:2]
        else:
            pq, psl = divmod(s - 1, SS)
            rhs = hs[pq][0:H, 2 + 2 * psl : 4 + 2 * psl]
        nc.tensor.matmul(
            psA[:, 2 * s : 2 * s + 2],
            Whh_sb[:],
            rhs,
            start=False,
            stop=True,
            skip_group_check=True,
        )
        nc.scalar.activation(
            hs[q][0:H, 2 + 2 * sl : 4 + 2 * sl], psA[:, 2 * s : 2 * s + 2], AFT.Tanh
        )

    nc.sync.dma_start(
        dr["h_lastT"].bitcast(f32r), hs[SEG - 1][0:H, 2 * SS : 2 * SS + 2]
    )

    # f32r -> bf16 conversion of hs for the FC matmuls; row H is the FC bias
    # ones-row (bf16 memset, left untouched by the partial-row cast).
    hsb = []
    for q in range(SEG):
        t = singles.tile([H + 1, 2 * SS], bf16, tag=f"hsb{q}")
        nc.vector.memset(t[:], 1.0)
        nc.vector.tensor_copy(
            t[0:H, :], hs[q].bitcast(f32)[0:H, 2 : 2 + 2 * SS]
        )
        hsb.append(t)

    # --- FC + log_softmax, 128-token tiles ---
    for h in range(SEG):
        for b in range(BL):
            lhsT = hsb[h][0 : H + 1, b : 2 * SS : 2]  # [101, 128]
            row0 = b * S + h * SS
            # pass A: exp-sums over 1024-wide psum groups
            stats = stats_p.tile([128, NG], f32)
            for j in range(NG):
                w = min(GW, V - j * GW)
                ps = fc_ps.tile([128, GW], f32, tag="fc")
                for k in range(0, w, 512):
                    kw = min(512, w - k)
                    nc.tensor.matmul(
                        ps[:, k : k + kw],
                        lhsT,
                        Wfcb[:, j * GW + k : j * GW + k + kw],
                        start=True,
                        stop=True,
                    )
                ex = expp.tile([128, GW], f32)
                nc.scalar.activation(
                    ex[:, 0:w], ps[:, 0:w], AFT.Exp, accum_out=stats[:, j : j + 1]
                )
            esum = small.tile([128, 1], f32)
            nc.vector.reduce_sum(esum[:], stats[:], axis=AX.X)
            lse = small.tile([128, 1], f32)
            nc.scalar.activation(lse[:], esum[:], AFT.Ln)
            # pass B: logits - lse, staged ~2MB output DMAs
            for g in range(NST):
                sw = min(SW, V - g * SW)
                stg = stage.tile([128, SW], f32)
                for jj in range((sw + GW - 1) // GW):
                    off = jj * GW
                    w = min(GW, sw - off)
                    ps = fc_ps.tile([128, GW], f32, tag="fc")
                    for k in range(0, w, 512):
                        kw = min(512, w - k)
                        nc.tensor.matmul(
                            ps[:, k : k + kw],
                            lhsT,
                            Wfcb[:, g * SW + off + k : g * SW + off + k + kw],
                            start=True,
                            stop=True,
                        )
                    nc.vector.tensor_scalar_sub(
                        stg[:, off : off + w], ps[:, 0:w], lse[:, 0:1]
                    )
                nc.gpsimd.dma_start(
                    dr["out_lp"][row0 : row0 + 128, g * SW : g * SW + sw],
                    stg[:, 0:sw],
                )


def build_module():
    nc = bacc.Bacc("TRN2", target_bir_lowering=False, debug=False)
    dr = {
        "xt_idx": nc.dram_tensor("xt_idx", [128, 4], i32, kind="ExternalInput").ap(),
        "h0T": nc.dram_tensor("h0T", [H, BL], f32, kind="ExternalInput").ap(),
        "W_aug": nc.dram_tensor("W_aug", [E + 2, H], f32, kind="ExternalInput").ap(),
        "W_hh": nc.dram_tensor("W_hh", [H, H], f32, kind="ExternalInput").ap(),
        "Wfc_aug": nc.dram_tensor(
            "Wfc_aug", [E + 1, V], f32, kind="ExternalInput"
        ).ap(),
        "emb": nc.dram_tensor("emb", [V, E], f32, kind="ExternalInput").ap(),
        "out_lp": nc.dram_tensor("out_lp", [TOK, V], f32, kind="ExternalOutput").ap(),
        "h_lastT": nc.dram_tensor(
            "h_lastT", [H, BL], f32, kind="ExternalOutput"
        ).ap(),
    }
    with tile.TileContext(nc) as tc:
        with ExitStack() as ctx:
            _emit(ctx, tc, dr)
    nc.compile()
    return nc


def make_in_maps(inputs):
    x = np.asarray(inputs["x"]).astype(np.int32)  # [16, 256]
    hidden = np.asarray(inputs["hidden"], dtype=np.float32)  # [16, 100]
    emb = np.ascontiguousarray(np.asarray(inputs["emb"], dtype=np.float32))
    W_ih = np.asarray(inputs["W_ih"], dtype=np.float32)
    W_hh = np.ascontiguousarray(np.asarray(inputs["W_hh"], dtype=np.float32))
    b_ih = np.asarray(inputs["b_ih"], dtype=np.float32)
    b_hh = np.asarray(inputs["b_hh"], dtype=np.float32)
    W_fc = np.asarray(inputs["W_fc"], dtype=np.float32)
    b_fc = np.asarray(inputs["b_fc"], dtype=np.float32)

    W_aug = np.ascontiguousarray(np.vstack([W_ih, b_ih[None, :], b_hh[None, :]]))
    Wfc_aug = np.ascontiguousarray(np.vstack([W_fc, b_fc[None, :]]))

    in_maps = []
    for c in range(NCORES):
        xb = x[BL * c : BL * (c + 1)]  # [2, 256]
        ti = np.ascontiguousarray(xb.T).reshape(TOK)  # time-major: i = 2*s + b
        xt_idx = np.ascontiguousarray(ti.reshape(4, 128).T)  # [p, j] = ti[128j + p]
        h0T = np.ascontiguousarray(hidden[BL * c : BL * (c + 1)].T)  # [100, 2]
        in_maps.append(
            {
                "xt_idx": xt_idx,
                "h0T": h0T,
                "W_aug": W_aug,
                "W_hh": W_hh,
                "Wfc_aug": Wfc_aug,
                "emb": emb,
            }
        )
    return in_maps


def assemble_outputs(results):
    log_probs = np.concatenate(
        [results[c]["out_lp"].reshape(BL, S, V) for c in range(NCORES)], axis=0
    )
    h_last = np.concatenate(
        [results[c]["h_lastT"].T for c in range(NCORES)], axis=0
    )
    return log_probs, h_last


_NC_CACHE = None


def kernel(**inputs):
    global _NC_CACHE
    if _NC_CACHE is None:
        _NC_CACHE = build_module()
    in_maps = make_in_maps(inputs)
    res = run_bass_kernel_spmd(_NC_CACHE, in_maps, core_ids=list(range(NCORES)))
    return assemble_outputs(res.results)
